# revision 5
# baseline (speedup 1.0000x reference)
"""Trainium2 Bass kernel for per-assignment batched linear (moe_routing).

Reference op: out[b, a, o] = sum_i weight[a, o, i] * x[b, a, i] + bias[a, o]
with B=4096, A=32, I=256, O=256, all float32.

Strategy: expert-parallel across the 8 NeuronCores — core c owns
assignments [4c, 4c+4). Each core's shard is x[:, 4c:4c+4, :] (16.8 MB),
weight[4c:4c+4] (1 MB), bias[4c:4c+4]; there is no cross-core traffic, so
no collectives are needed and the gather is a host-side concatenate.

Device layout: the contraction dim I must sit on SBUF partitions for the
TensorEngine, so the host pre-transposes x to [A, I, B] (free: tokens) and
weight to lhsT tiles [ki, oj].  Per (a, o-chunk) the kernel runs
weight-stationary matmuls out[oj, tok] += wT.T @ xT with a 512-wide moving
dim (float32r runs at 1 cycle/row there), accumulating the two I-chunks in
PSUM.  Bias is per-partition in this orientation, so it is fused into the
PSUM->SBUF eviction (scalar.activation / vector.tensor_scalar_add).  The
output leaves the device as [A_loc, O, B] and the host transposes back.
"""

import os

import numpy as np

P = 128
B, A, I, O = 4096, 32, 256, 256
NCORES = 8
A_LOC = A // NCORES  # assignments per core
KC = I // P  # contraction chunks
OC = O // P  # output-row chunks
G = 512  # matmul moving free dim
NG = B // G

_NC_CACHE = None
LAST_RESULT = None  # BassKernelResults of the most recent run (for test harnesses)


def _build():
    import concourse.tile as tile
    from concourse import bacc, mybir

    nc = bacc.Bacc(
        "TRN2",
        target_bir_lowering=False,
        debug=False,
        num_devices=NCORES,
    )
    f32 = mybir.dt.float32
    f32r = mybir.dt.float32r
    ident = mybir.ActivationFunctionType.Identity

    x_ext = nc.dram_tensor("x", [A_LOC, KC, P, B], f32r, kind="ExternalInput").ap()
    w_ext = nc.dram_tensor(
        "w", [P, A_LOC * KC * OC * P], f32r, kind="ExternalInput"
    ).ap()
    b_ext = nc.dram_tensor("b", [P, A_LOC * OC], f32, kind="ExternalInput").ap()
    out_ext = nc.dram_tensor(
        "out", [A_LOC, OC, P, B], f32, kind="ExternalOutput"
    ).ap()

    with tile.TileContext(nc) as tc:
        with (
            tc.tile_pool(name="xp", bufs=2) as xp,
            tc.tile_pool(name="wp", bufs=1) as wp,
            tc.tile_pool(name="bp", bufs=1) as bp,
            tc.tile_pool(name="op", bufs=4) as op_,
            tc.tile_pool(name="pp", bufs=8, space="PSUM") as pp,
        ):
            w_sb = wp.tile([P, A_LOC * KC * OC * P], f32r)
            nc.sync.dma_start(out=w_sb[:], in_=w_ext[:])
            b_sb = bp.tile([P, A_LOC * OC], f32)
            nc.sync.dma_start(out=b_sb[:], in_=b_ext[:])

            for a in range(A_LOC):
                xt = xp.tile([P, KC, B], f32r, tag="x")
                nc.sync.dma_start(out=xt[:], in_=x_ext[a].transpose([1, 0, 2]))
                for o in range(OC):
                    out_sb = op_.tile([P, B], f32, tag="o")
                    bias_ap = b_sb[:, a * OC + o : a * OC + o + 1]
                    for g in range(NG):
                        ps = pp.tile([P, G], f32)
                        for k in range(KC):
                            col = ((a * KC + k) * OC + o) * P
                            nc.tensor.matmul(
                                ps[:],
                                w_sb[:, col : col + P],
                                xt[:, k, g * G : (g + 1) * G],
                                start=(k == 0),
                                stop=(k == KC - 1),
                            )
                        dst = out_sb[:, g * G : (g + 1) * G]
                        if g % 2 == 0:
                            nc.scalar.activation(dst, ps[:], ident, bias=bias_ap)
                        else:
                            nc.vector.tensor_scalar_add(dst, ps[:], bias_ap)
                    nc.scalar.dma_start(out=out_ext[a, o], in_=out_sb[:])

    nc.compile()
    return nc


def _get_nc():
    global _NC_CACHE
    if _NC_CACHE is None:
        _NC_CACHE = _build()
    return _NC_CACHE


def kernel(x, weight, bias):
    from concourse.bass_utils import run_bass_kernel_spmd

    global LAST_RESULT

    x = np.ascontiguousarray(np.asarray(x), dtype=np.float32)  # [B, A, I]
    weight = np.ascontiguousarray(np.asarray(weight), dtype=np.float32)  # [A, O, I]
    bias = np.ascontiguousarray(np.asarray(bias), dtype=np.float32)  # [A, O]

    # x -> [A, I, B] -> per-core [A_LOC, KC, P, B]
    xT = np.ascontiguousarray(x.transpose(1, 2, 0))
    xT = xT.reshape(NCORES, A_LOC, KC, P, B)

    # weight[aG, o*P+oj, k*P+ki] -> w[c][ki, ((a*KC+k)*OC+o)*P + oj]
    w = weight.reshape(NCORES, A_LOC, OC, P, KC, P)  # [c, a, o, oj, k, ki]
    w = np.ascontiguousarray(w.transpose(0, 5, 1, 4, 2, 3))  # [c, ki, a, k, o, oj]
    w = w.reshape(NCORES, P, A_LOC * KC * OC * P)

    # bias[aG, o*P+oj] -> b[c][oj, a*OC+o]
    bb = bias.reshape(NCORES, A_LOC, OC, P)  # [c, a, o, oj]
    bb = np.ascontiguousarray(bb.transpose(0, 3, 1, 2)).reshape(
        NCORES, P, A_LOC * OC
    )

    nc = _get_nc()
    in_maps = [{"x": xT[c], "w": w[c], "b": bb[c]} for c in range(NCORES)]
    res = run_bass_kernel_spmd(nc, in_maps, core_ids=list(range(NCORES)))
    LAST_RESULT = res

    outs = [np.asarray(res.results[c]["out"]) for c in range(NCORES)]
    out = np.concatenate(outs, axis=0)  # [A, OC, P, B]
    out = out.reshape(A, O, B).transpose(2, 0, 1)  # [B, A, O]
    return np.ascontiguousarray(out)


if __name__ == "__main__":
    rng = np.random.default_rng(0)
    x = rng.standard_normal((B, A, I), dtype=np.float32)
    weight = rng.standard_normal((A, O, I), dtype=np.float32) / np.sqrt(I)
    bias = rng.standard_normal((A, O), dtype=np.float32)
    out = kernel(x, weight, bias)
    ref = np.einsum("aoi,bai->bao", weight, x) + bias
    err = np.abs(out - ref).max() / np.abs(ref).max()
    print("max-rel-err vs local numpy ref:", err)


# revision 6
# speedup vs baseline: 1.3866x; 1.3866x over previous
"""Trainium2 Bass kernel for per-assignment batched linear (moe_routing).

Reference op: out[b, a, o] = sum_i weight[a, o, i] * x[b, a, i] + bias[a, o]
with B=4096, A=32, I=256, O=256, all float32.

Strategy: expert-parallel across the 8 NeuronCores — core c owns
assignments [4c, 4c+4). Each core's shard is x[:, 4c:4c+4, :], weight and
bias slices for those assignments; there is no cross-core traffic, so no
collectives are needed and the gather is a host-side concatenate.

Device layout: the contraction dim I must sit on SBUF partitions for the
TensorEngine, so the host pre-transposes x to [A, I, B] (free: tokens) and
weight to lhsT tiles [ki, oj].  Per (a, o-chunk) the kernel runs
weight-stationary matmuls out[oj, tok] += wT.T @ xT with a 512-wide moving
dim, accumulating the two I-chunks in PSUM (always fp32).  Bias is
per-partition in this orientation, so it is fused into the PSUM->SBUF
eviction (scalar.activation Identity / vector.tensor_scalar_add, both take
a [128,1] per-partition bias AP).  The output leaves the device as
[A_loc, O, B] and the host transposes back.

The kernel is memory-bound (per-core DMA ~34.6 MB fp32 vs ~27 us of
TensorE work), so the wire dtype is the main lever.  IO_DTYPE selects it:
  "f32r"  - full fp32 storage; fp32r matmul (reduced-precision fp32 mode
            that streams 1 column/cycle at N>=256; measured rel err 1.3e-4)
  "bf16"  - x/weight and the output cross HBM as bf16, accumulate fp32,
            bias added in fp32 (measured rel err ~3e-3); ~2x fewer bytes
"""

import numpy as np

P = 128
B, A, I, O = 4096, 32, 256, 256
NCORES = 8
A_LOC = A // NCORES  # assignments per core
KC = I // P  # contraction chunks
OC = O // P  # output-row chunks
G = 512  # matmul moving free dim
NG = B // G

IO_DTYPE = "bf16"

_NC_CACHE = {}
LAST_RESULT = None  # BassKernelResults of the most recent run (for test harnesses)


def _build(io_dtype):
    import concourse.tile as tile
    from concourse import bacc, mybir

    nc = bacc.Bacc(
        "TRN2",
        target_bir_lowering=False,
        debug=False,
        num_devices=NCORES,
    )
    f32 = mybir.dt.float32
    ident = mybir.ActivationFunctionType.Identity
    in_dt = mybir.dt.float32r if io_dtype == "f32r" else mybir.dt.bfloat16
    out_dt = f32 if io_dtype == "f32r" else mybir.dt.bfloat16

    x_ext = nc.dram_tensor("x", [A_LOC, KC, P, B], in_dt, kind="ExternalInput").ap()
    w_ext = nc.dram_tensor(
        "w", [P, A_LOC * KC * OC * P], in_dt, kind="ExternalInput"
    ).ap()
    b_ext = nc.dram_tensor("b", [P, A_LOC * OC], f32, kind="ExternalInput").ap()
    out_ext = nc.dram_tensor(
        "out", [A_LOC, OC, P, B], out_dt, kind="ExternalOutput"
    ).ap()

    with tile.TileContext(nc) as tc:
        with (
            tc.tile_pool(name="xp", bufs=2) as xp,
            tc.tile_pool(name="wp", bufs=1) as wp,
            tc.tile_pool(name="bp", bufs=1) as bp,
            tc.tile_pool(name="op", bufs=2) as op_,
            tc.tile_pool(name="pp", bufs=8, space="PSUM") as pp,
        ):
            w_sb = wp.tile([P, A_LOC * KC * OC * P], in_dt)
            nc.sync.dma_start(out=w_sb[:], in_=w_ext[:])
            b_sb = bp.tile([P, A_LOC * OC], f32)
            nc.sync.dma_start(out=b_sb[:], in_=b_ext[:])

            for a in range(A_LOC):
                xt = xp.tile([P, KC, B], in_dt, tag="x")
                nc.sync.dma_start(out=xt[:], in_=x_ext[a].transpose([1, 0, 2]))
                out_sb = op_.tile([P, OC, B], out_dt, tag="o")
                for o in range(OC):
                    bias_ap = b_sb[:, a * OC + o : a * OC + o + 1]
                    for g in range(NG):
                        ps = pp.tile([P, G], f32)
                        for k in range(KC):
                            col = ((a * KC + k) * OC + o) * P
                            nc.tensor.matmul(
                                ps[:],
                                w_sb[:, col : col + P],
                                xt[:, k, g * G : (g + 1) * G],
                                start=(k == 0),
                                stop=(k == KC - 1),
                            )
                        dst = out_sb[:, o, g * G : (g + 1) * G]
                        if g % 2 == 0:
                            nc.scalar.activation(dst, ps[:], ident, bias=bias_ap)
                        else:
                            nc.vector.tensor_scalar_add(dst, ps[:], bias_ap)
                nc.scalar.dma_start(
                    out=out_ext[a].transpose([1, 0, 2]), in_=out_sb[:]
                )

    nc.compile()
    return nc


def _get_nc(io_dtype):
    if io_dtype not in _NC_CACHE:
        _NC_CACHE[io_dtype] = _build(io_dtype)
    return _NC_CACHE[io_dtype]


def kernel(x, weight, bias):
    import ml_dtypes
    from concourse.bass_utils import run_bass_kernel_spmd

    global LAST_RESULT

    x = np.ascontiguousarray(np.asarray(x), dtype=np.float32)  # [B, A, I]
    weight = np.ascontiguousarray(np.asarray(weight), dtype=np.float32)  # [A, O, I]
    bias = np.ascontiguousarray(np.asarray(bias), dtype=np.float32)  # [A, O]

    np_in = np.float32 if IO_DTYPE == "f32r" else ml_dtypes.bfloat16

    # x -> [A, I, B] -> per-core [A_LOC, KC, P, B]
    xT = np.ascontiguousarray(x.transpose(1, 2, 0)).astype(np_in)
    xT = xT.reshape(NCORES, A_LOC, KC, P, B)

    # weight[aG, o*P+oj, k*P+ki] -> w[c][ki, ((a*KC+k)*OC+o)*P + oj]
    w = weight.reshape(NCORES, A_LOC, OC, P, KC, P)  # [c, a, o, oj, k, ki]
    w = np.ascontiguousarray(w.transpose(0, 5, 1, 4, 2, 3)).astype(np_in)
    w = w.reshape(NCORES, P, A_LOC * KC * OC * P)

    # bias[aG, o*P+oj] -> b[c][oj, a*OC+o]
    bb = bias.reshape(NCORES, A_LOC, OC, P)  # [c, a, o, oj]
    bb = np.ascontiguousarray(bb.transpose(0, 3, 1, 2)).reshape(
        NCORES, P, A_LOC * OC
    )

    nc = _get_nc(IO_DTYPE)
    in_maps = [{"x": xT[c], "w": w[c], "b": bb[c]} for c in range(NCORES)]
    res = run_bass_kernel_spmd(nc, in_maps, core_ids=list(range(NCORES)))
    LAST_RESULT = res

    outs = [np.asarray(res.results[c]["out"]) for c in range(NCORES)]
    out = np.concatenate(outs, axis=0)  # [A, OC, P, B]
    out = out.astype(np.float32).reshape(A, O, B).transpose(2, 0, 1)  # [B, A, O]
    return np.ascontiguousarray(out)


if __name__ == "__main__":
    rng = np.random.default_rng(0)
    x = rng.standard_normal((B, A, I), dtype=np.float32)
    weight = rng.standard_normal((A, O, I), dtype=np.float32) / np.sqrt(I)
    bias = rng.standard_normal((A, O), dtype=np.float32)
    out = kernel(x, weight, bias)
    ref = np.einsum("aoi,bai->bao", weight, x) + bias
    err = np.abs(out - ref).max() / np.abs(ref).max()
    print("max-rel-err vs local numpy ref:", err)


# revision 8
# speedup vs baseline: 1.8098x; 1.3053x over previous
"""Trainium2 Bass kernel for per-assignment batched linear (moe_routing).

Reference op: out[b, a, o] = sum_i weight[a, o, i] * x[b, a, i] + bias[a, o]
with B=4096, A=32, I=256, O=256, all float32.

Strategy: expert-parallel across the 8 NeuronCores — core c owns
assignments [4c, 4c+4). Each core's shard is x[:, 4c:4c+4, :], weight and
bias slices for those assignments; there is no cross-core traffic, so no
collectives are needed and the gather is a host-side concatenate.

Device layout: the contraction dim I must sit on SBUF partitions for the
TensorEngine, so the host pre-transposes x to [A, I, B] (free: tokens) and
weight to lhsT tiles [ki, oj].  Per (a, o-chunk) the kernel runs
weight-stationary matmuls out[oj, tok] += wT.T @ xT with a 512-wide moving
dim, accumulating the two I-chunks in PSUM (always fp32).  Bias is
per-partition in this orientation, so it is fused into the PSUM->SBUF
eviction (scalar.activation Identity / vector.tensor_scalar_add, both take
a [128,1] per-partition bias AP).  The output leaves the device as
[A_loc, O, B] and the host transposes back.

The kernel is memory-bound (per-core DMA ~34.6 MB fp32 vs ~27 us of
TensorE work), so the wire dtype is the main lever.  IO_DTYPE selects it:
  "f32r"  - full fp32 storage; fp32r matmul (reduced-precision fp32 mode
            that streams 1 column/cycle at N>=256; measured rel err 1.3e-4)
  "bf16"  - x/weight and the output cross HBM as bf16, accumulate fp32,
            bias added in fp32 (measured rel err ~3e-3); ~2x fewer bytes
"""

import numpy as np

P = 128
B, A, I, O = 4096, 32, 256, 256
NCORES = 8
A_LOC = A // NCORES  # assignments per core
KC = I // P  # contraction chunks
OC = O // P  # output-row chunks
G = 512  # matmul moving free dim
NG = B // G

IO_DTYPE = "bf16"

_NC_CACHE = {}
LAST_RESULT = None  # BassKernelResults of the most recent run (for test harnesses)


def _build(io_dtype):
    import concourse.tile as tile
    from concourse import bacc, mybir

    nc = bacc.Bacc(
        "TRN2",
        target_bir_lowering=False,
        debug=False,
        num_devices=NCORES,
    )
    f32 = mybir.dt.float32
    ident = mybir.ActivationFunctionType.Identity
    in_dt = mybir.dt.float32r if io_dtype == "f32r" else mybir.dt.bfloat16
    out_dt = f32 if io_dtype == "f32r" else mybir.dt.bfloat16

    x_ext = nc.dram_tensor("x", [A_LOC, KC, P, B], in_dt, kind="ExternalInput").ap()
    w_ext = nc.dram_tensor(
        "w", [P, A_LOC * KC * OC * P], in_dt, kind="ExternalInput"
    ).ap()
    b_ext = nc.dram_tensor("b", [P, A_LOC * OC], f32, kind="ExternalInput").ap()
    out_ext = nc.dram_tensor(
        "out", [A_LOC, OC, P, B], out_dt, kind="ExternalOutput"
    ).ap()

    xp_bufs = NCORES if io_dtype == "bf16" else 4  # x tiles resident
    op_bufs = 4 if io_dtype == "bf16" else 2
    E = 2 * G  # eviction width: one 2-bank PSUM tile

    with tile.TileContext(nc) as tc:
        with (
            tc.tile_pool(name="xp", bufs=xp_bufs) as xp,
            tc.tile_pool(name="wp", bufs=1) as wp,
            tc.tile_pool(name="bp", bufs=1) as bp,
            tc.tile_pool(name="op", bufs=op_bufs) as op_,
            tc.tile_pool(name="pp", bufs=4, space="PSUM") as pp,
        ):
            w_sb = wp.tile([P, A_LOC * KC * OC * P], in_dt)
            nc.gpsimd.dma_start(out=w_sb[:], in_=w_ext[:])
            b_sb = bp.tile([P, A_LOC * OC], f32)
            nc.gpsimd.dma_start(out=b_sb[:], in_=b_ext[:])

            for a in range(A_LOC):
                xt = []
                for k in range(KC):
                    t = xp.tile([P, B], in_dt, tag="x")
                    nc.sync.dma_start(out=t[:], in_=x_ext[a, k])
                    xt.append(t)
                for o in range(OC):
                    out_sb = op_.tile([P, B], out_dt, tag="o")
                    bias_ap = b_sb[:, a * OC + o : a * OC + o + 1]
                    for gg in range(NG // 2):
                        ps = pp.tile([P, E], f32)
                        for j in range(2):
                            g = gg * 2 + j
                            for k in range(KC):
                                col = ((a * KC + k) * OC + o) * P
                                nc.tensor.matmul(
                                    ps[:, j * G : (j + 1) * G],
                                    w_sb[:, col : col + P],
                                    xt[k][:, g * G : (g + 1) * G],
                                    start=(k == 0),
                                    stop=(k == KC - 1),
                                )
                        dst = out_sb[:, gg * E : (gg + 1) * E]
                        if gg % 2 == 0:
                            nc.scalar.activation(dst, ps[:], ident, bias=bias_ap)
                        else:
                            nc.vector.tensor_scalar_add(dst, ps[:], bias_ap)
                    nc.scalar.dma_start(out=out_ext[a, o], in_=out_sb[:])

    nc.compile()
    return nc


def _get_nc(io_dtype):
    if io_dtype not in _NC_CACHE:
        _NC_CACHE[io_dtype] = _build(io_dtype)
    return _NC_CACHE[io_dtype]


def kernel(x, weight, bias):
    import os

    import ml_dtypes
    from concourse.bass_utils import run_bass_kernel_spmd

    global LAST_RESULT

    # Tracing needs an NTFF hook this container only has when the harness
    # (test.py) installs it; suppress it unless explicitly opted in so a
    # stray BASS_TRACE env can't break the run.
    if os.environ.get("KERNEL_TRACE") != "1":
        os.environ["BASS_NEVER_TRACE"] = "1"

    x = np.ascontiguousarray(np.asarray(x), dtype=np.float32)  # [B, A, I]
    weight = np.ascontiguousarray(np.asarray(weight), dtype=np.float32)  # [A, O, I]
    bias = np.ascontiguousarray(np.asarray(bias), dtype=np.float32)  # [A, O]

    np_in = np.float32 if IO_DTYPE == "f32r" else ml_dtypes.bfloat16

    # x -> [A, I, B] -> per-core [A_LOC, KC, P, B]
    xT = np.ascontiguousarray(x.transpose(1, 2, 0)).astype(np_in)
    xT = xT.reshape(NCORES, A_LOC, KC, P, B)

    # weight[aG, o*P+oj, k*P+ki] -> w[c][ki, ((a*KC+k)*OC+o)*P + oj]
    w = weight.reshape(NCORES, A_LOC, OC, P, KC, P)  # [c, a, o, oj, k, ki]
    w = np.ascontiguousarray(w.transpose(0, 5, 1, 4, 2, 3)).astype(np_in)
    w = w.reshape(NCORES, P, A_LOC * KC * OC * P)

    # bias[aG, o*P+oj] -> b[c][oj, a*OC+o]
    bb = bias.reshape(NCORES, A_LOC, OC, P)  # [c, a, o, oj]
    bb = np.ascontiguousarray(bb.transpose(0, 3, 1, 2)).reshape(
        NCORES, P, A_LOC * OC
    )

    nc = _get_nc(IO_DTYPE)
    in_maps = [{"x": xT[c], "w": w[c], "b": bb[c]} for c in range(NCORES)]
    res = run_bass_kernel_spmd(nc, in_maps, core_ids=list(range(NCORES)))
    LAST_RESULT = res

    outs = [np.asarray(res.results[c]["out"]) for c in range(NCORES)]
    out = np.concatenate(outs, axis=0)  # [A, OC, P, B]
    out = out.astype(np.float32).reshape(A, O, B).transpose(2, 0, 1)  # [B, A, O]
    return np.ascontiguousarray(out)


if __name__ == "__main__":
    rng = np.random.default_rng(0)
    x = rng.standard_normal((B, A, I), dtype=np.float32)
    weight = rng.standard_normal((A, O, I), dtype=np.float32) / np.sqrt(I)
    bias = rng.standard_normal((A, O), dtype=np.float32)
    out = kernel(x, weight, bias)
    ref = np.einsum("aoi,bai->bao", weight, x) + bias
    err = np.abs(out - ref).max() / np.abs(ref).max()
    print("max-rel-err vs local numpy ref:", err)
